# revision 7
# baseline (speedup 1.0000x reference)
"""Pairwise cosine similarity on 8 Trainium2 NeuronCores.

Computes sim[n, m] = <x_n, y_m> / (||x_n|| * ||y_m||) for
input1 [8192, 128], input2 [8192, 128] -> out [8192, 8192] (fp32 API).

Sharding: input1 rows split 8 ways (1024 rows/core); input2 replicated.
Each core computes one [1024, 8192] output stripe; host concatenates.

Precision plan (checker budget: rel_err < 2e-2 vs absmax):
host casts inputs to bf16 (round-to-nearest) and upcasts the bf16 output
stripe to fp32. Device math: bf16 matmul with fp32 PSUM accumulation,
inv-norms via Sqrt + reciprocal_approx_fast (~51 ULP). Measured stack
lands ~1e-3 rel err -- an order under budget -- while halving both HBM
streams (loads 2.25 MB, stores 16.8 MB per core).

Per-core dataflow (all operands pre-transposed to [d, rows] layout by
DMA xbar transpose on load -- zero PE transposes, no fp32 operands):

  xT   <- dma_transpose(x)      [128, 1024] bf16   (ACT HWDGE ring)
  yT_c <- dma_transpose(y_c)    [128, 2048] bf16 per column chunk
  sq   = yT_c * yT_c            (GpSimd, SBUF)
  n2   = ones.T @ sq            (PE; broadcasts ||y_m||^2 to all partitions)
  nrm  = sqrt(n2)               (ACT, fused with the PSUM->SBUF drain)
  inv  = 1/nrm                  (DVE reciprocal_approx_fast)
  yTn  = yT_c * bf16(inv)       (GpSimd)
  ps   = xTn_b.T @ yTn          (PE, N=512 per fp32 PSUM bank)
  ob   = bf16(ps)               (drain: DVE tensor_copy / ACT copy, alternating)
  out  <- ob                    (SP HWDGE ring, 512 KB stores)

The drain is the scarce resource (PSUM reads are 1x-rate on both DVE and
ACT); everything else is placed to keep those two queues copy-only.
"""

import numpy as np
import ml_dtypes

import concourse.bass as bass
import concourse.tile as tile
from concourse import bacc, mybir
from concourse.bass_utils import run_bass_kernel_spmd

N_CORES = 8
D = 128          # feature dim == contraction partitions
P = 128          # SBUF partitions
NT = 512         # matmul free dim (one fp32 PSUM bank)
CHUNK = 2048     # corpus columns per outer chunk
MMCOLS = 1024    # PSUM tile columns (2 banks, 2 matmuls, 1 drain)

F32 = mybir.dt.float32
BF16 = mybir.dt.bfloat16


def build_nc(rows_per_core: int, corpus_rows: int) -> bass.Bass:
    # Bacc compile() splits multi-sem waits into event-semaphore
    # instructions where an instruction can carry only one wait.
    nc = bacc.Bacc(None)

    x = nc.dram_tensor("x", [rows_per_core, D], BF16, kind="ExternalInput")
    y = nc.dram_tensor("y", [corpus_rows, D], BF16, kind="ExternalInput")
    out = nc.dram_tensor(
        "out", [rows_per_core, corpus_rows], BF16, kind="ExternalOutput"
    )

    nbx = rows_per_core // P          # x row-blocks (8)
    nchunk = corpus_rows // CHUNK     # corpus column chunks (4)

    with tile.TileContext(nc) as tc:
        with (
            tc.tile_pool(name="const", bufs=1) as constp,
            tc.tile_pool(name="xp", bufs=1) as xp,
            tc.tile_pool(name="yt", bufs=2) as ytp,
            tc.tile_pool(name="sq", bufs=2) as sqp,
            tc.tile_pool(name="nrm", bufs=2) as nrmp,
            tc.tile_pool(name="inv", bufs=2) as invp,
            tc.tile_pool(name="yn", bufs=2) as ynp,
            tc.tile_pool(name="obuf", bufs=4) as obufp,
            tc.tile_pool(name="mm", bufs=3, space=bass.MemorySpace.PSUM) as mpsum,
            tc.tile_pool(name="nm", bufs=1, space=bass.MemorySpace.PSUM) as npsum,
        ):
            ones = constp.tile([P, P], BF16)
            nc.gpsimd.memset(ones[:], 1.0)

            # PE warm-up: dummy bf16 matmuls under the initial loads so the
            # HAM clock gate opens (1.2 -> 2.4 GHz) before real matmuls.
            wt = constp.tile([P, NT], BF16)
            nc.gpsimd.memset(wt[:], 0.0)
            wps = mpsum.tile([P, MMCOLS], F32, tag="ps")
            for i in range(12):
                nc.tensor.matmul(
                    wps[:, (i % 2) * NT : (i % 2) * NT + NT],
                    wt[:, :P],
                    wt[:],
                    start=True,
                    stop=True,
                )

            drain_rr = [0]

            # Load a [cols, D] row-range of src transposed into a bf16
            # [128, cols] SBUF tile (xbar transpose, ACT HWDGE ring), then
            # produce its inverse-norm broadcast and the normalized operand.
            def prep_chunk(src, r0, cols):
                tT = ytp.tile([P, CHUNK], BF16, tag="yt")
                nc.scalar.dma_start(
                    out=tT[:, :cols], in_=src[r0 : r0 + cols, :], transpose=True
                )
                sq = sqp.tile([P, CHUNK], BF16, tag="sq")
                nc.gpsimd.tensor_mul(sq[:, :cols], tT[:, :cols], tT[:, :cols])
                nrm = nrmp.tile([P, CHUNK], F32, tag="nrm")
                for h in range(0, cols, MMCOLS):
                    hc = min(MMCOLS, cols - h)
                    nps = npsum.tile([P, MMCOLS], F32)
                    for j in range(0, hc, NT):
                        nc.tensor.matmul(
                            nps[:, j : j + NT],
                            ones[:],
                            sq[:, h + j : h + j + NT],
                            start=True,
                            stop=True,
                        )
                    # ||.|| broadcast to every partition, fused PSUM drain.
                    nc.scalar.sqrt(nrm[:, h : h + hc], nps[:, :hc])
                inv = invp.tile([P, CHUNK], F32, tag="inv")
                nc.vector.reciprocal_approx_fast(inv[:, :cols], nrm[:, :cols])
                invb = sqp.tile([P, CHUNK], BF16, tag="invb")
                nc.gpsimd.tensor_copy(invb[:, :cols], inv[:, :cols])
                tn = ynp.tile([P, CHUNK], BF16, tag="yn")
                nc.gpsimd.tensor_mul(tn[:, :cols], tT[:, :cols], invb[:, :cols])
                return tn

            xTn = prep_chunk(x[:], 0, rows_per_core)
            yTn = prep_chunk(y[:], 0, CHUNK)

            for c in range(nchunk):
                col0 = c * CHUNK
                yTn_next = None
                for b in range(nbx):
                    if b == 1 and c + 1 < nchunk:
                        # Next chunk's prep rides ahead of this chunk's
                        # drain flood on every queue.
                        yTn_next = prep_chunk(y[:], (c + 1) * CHUNK, CHUNK)
                    lhs = xTn[:, b * P : (b + 1) * P]
                    ob = obufp.tile([P, CHUNK], BF16, tag="ob")
                    for h in range(0, CHUNK, MMCOLS):
                        ps = mpsum.tile([P, MMCOLS], F32)
                        for j in range(0, MMCOLS, NT):
                            nc.tensor.matmul(
                                ps[:, j : j + NT],
                                lhs,
                                yTn[:, h + j : h + j + NT],
                                start=True,
                                stop=True,
                            )
                        dst = ob[:, h : h + MMCOLS]
                        # Alternate the PSUM->SBUF drain between DVE and ACT.
                        if drain_rr[0] % 2 == 0:
                            nc.vector.tensor_copy(dst, ps[:])
                        else:
                            nc.scalar.copy(dst, ps[:])
                        drain_rr[0] += 1
                    nc.sync.dma_start(
                        out=out[b * P : (b + 1) * P, col0 : col0 + CHUNK],
                        in_=ob[:],
                    )
                if yTn_next is not None:
                    yTn = yTn_next

    nc.finalize()
    return nc


_NC_CACHE: dict[tuple[int, int], bass.Bass] = {}


def run_spmd(input1: np.ndarray, input2: np.ndarray, **kwargs):
    """Shard, run on 8 cores, gather. Returns (output, BassKernelResults)."""
    x_bf = np.asarray(input1, dtype=np.float32).astype(ml_dtypes.bfloat16)
    y_bf = np.ascontiguousarray(
        np.asarray(input2, dtype=np.float32).astype(ml_dtypes.bfloat16)
    )
    n, d = x_bf.shape
    m, d2 = y_bf.shape
    assert d == D and d2 == D and n % N_CORES == 0
    rows = n // N_CORES

    key = (rows, m)
    if key not in _NC_CACHE:
        _NC_CACHE[key] = build_nc(rows, m)
    nc = _NC_CACHE[key]

    in_maps = [
        {"x": np.ascontiguousarray(x_bf[c * rows : (c + 1) * rows]), "y": y_bf}
        for c in range(N_CORES)
    ]
    res = run_bass_kernel_spmd(nc, in_maps, core_ids=list(range(N_CORES)), **kwargs)
    out = np.concatenate(
        [res.results[c]["out"].astype(np.float32) for c in range(N_CORES)], axis=0
    )
    return out, res


def kernel(input1: np.ndarray, input2: np.ndarray) -> np.ndarray:
    return run_spmd(input1, input2)[0]


# revision 8
# speedup vs baseline: 1.1601x; 1.1601x over previous
"""Pairwise cosine similarity on 8 Trainium2 NeuronCores.

Computes sim[n, m] = <x_n, y_m> / (||x_n|| * ||y_m||) for
input1 [8192, 128], input2 [8192, 128] -> out [8192, 8192] (fp32 API).

Sharding: input1 rows split 8 ways (1024 rows/core); input2 replicated.
Each core computes one [1024, 8192] output stripe; host concatenates.

Precision plan (checker budget: rel_err < 2e-2 vs absmax): host casts
inputs to bf16 (round-to-nearest) and upcasts the bf16 output stripe to
fp32. Device math: bf16 matmuls with fp32 PSUM accumulation; inv-norms
via reciprocal_approx_fast (~51 ULP) + ACT Sqrt. Measured ~7e-3 rel
err -- well inside budget -- and both HBM streams are 2-byte
(loads 2.25 MB + stores 16.8 MB per core ~= 53 us HBM floor).

Per-core dataflow ([d, rows] operand layout comes straight from DMA
xbar transpose on load -- zero PE transposes, zero fp32 matmul operands):

  xT    <- dma_transpose(x)    [128, 1024] bf16  (ACT HWDGE ring, raw)
  x_nat <- x                   [128, 8, 128]     (GpSimd SWDGE ring)
  invx  = 1/sqrt(reduce(x_nat^2))  [128, 8] fp32 per-partition row norms
  yT_c  <- dma_transpose(y_c)  [128, cols] bf16 per column chunk
  sq    = yT_c * yT_c              (GpSimd TT, off critical queues)
  n2    = ones.T @ sq              (PE: broadcasts ||y_m||^2 down partitions)
  in2   = recip_approx(n2)         (DVE, fused PSUM drain)
  invb  = bf16(sqrt(in2))          (ACT, fused fp32->bf16 cast)
  yTn   = yT_c * invb              (GpSimd TT)
  ps    = xT_b.T @ yTn             (PE, N=512 per fp32 PSUM bank)
  ob    = bf16(ps * invx[:, b])    (drain + x-norm fold: DVE tensor_scalar
                                    / ACT activation-scale, alternating)
  out   <- ob                      (SP HWDGE ring, 512 KB stores)

The PSUM->SBUF drain is the scarce resource (1x rate on both DVE and
ACT); every other op is placed to keep those two queues drain-only.
"""

import numpy as np
import ml_dtypes

import concourse.bass as bass
import concourse.tile as tile
from concourse import bacc, mybir
from concourse.bass_utils import run_bass_kernel_spmd

N_CORES = 8
D = 128          # feature dim == contraction partitions
P = 128          # SBUF partitions
NT = 512         # matmul free dim (one fp32 PSUM bank)
CHUNK = 2048     # max corpus columns per outer chunk
MMCOLS = 1024    # PSUM tile columns (2 banks, 2 matmuls, 1 drain)

F32 = mybir.dt.float32
BF16 = mybir.dt.bfloat16
ACTF = mybir.ActivationFunctionType


def build_nc(rows_per_core: int, corpus_rows: int) -> bass.Bass:
    # Bacc compile() splits multi-sem waits into event-semaphore
    # instructions where an instruction can carry only one wait.
    nc = bacc.Bacc(None)

    x = nc.dram_tensor("x", [rows_per_core, D], BF16, kind="ExternalInput")
    y = nc.dram_tensor("y", [corpus_rows, D], BF16, kind="ExternalInput")
    out = nc.dram_tensor(
        "out", [rows_per_core, corpus_rows], BF16, kind="ExternalOutput"
    )

    nbx = rows_per_core // P          # x row-blocks (8)
    # Small first chunk starts the store pipeline early; small last chunk
    # shortens the drain tail after the final matmul.
    if corpus_rows >= 4 * CHUNK:
        half = CHUNK // 2
        nfull = (corpus_rows - 2 * half) // CHUNK
        chunk_cols = [half] + [CHUNK] * nfull + [half]
    else:
        chunk_cols = [CHUNK] * (corpus_rows // CHUNK)
    assert sum(chunk_cols) == corpus_rows
    chunk_starts = [sum(chunk_cols[:i]) for i in range(len(chunk_cols))]

    with tile.TileContext(nc) as tc:
        with (
            tc.tile_pool(name="const", bufs=1) as constp,
            tc.tile_pool(name="xp", bufs=1) as xp,
            tc.tile_pool(name="yt", bufs=2) as ytp,
            tc.tile_pool(name="sq", bufs=2) as sqp,
            tc.tile_pool(name="in2", bufs=2) as in2p,
            tc.tile_pool(name="invb", bufs=2) as invbp,
            tc.tile_pool(name="yn", bufs=2) as ynp,
            tc.tile_pool(name="obuf", bufs=4) as obufp,
            tc.tile_pool(name="mm", bufs=3, space=bass.MemorySpace.PSUM) as mpsum,
            tc.tile_pool(name="nm", bufs=1, space=bass.MemorySpace.PSUM) as npsum,
        ):
            ones = constp.tile([P, P], BF16)
            nc.gpsimd.memset(ones[:], 1.0)

            # PE warm-up: dummy bf16 matmuls under the initial loads so the
            # HAM clock gate opens (1.2 -> 2.4 GHz) before real matmuls.
            wt = constp.tile([P, NT], BF16)
            nc.gpsimd.memset(wt[:], 0.0)
            wps = mpsum.tile([P, MMCOLS], F32, tag="ps")
            for i in range(12):
                nc.tensor.matmul(
                    wps[:, (i % 2) * NT : (i % 2) * NT + NT],
                    wt[:, :P],
                    wt[:],
                    start=True,
                    stop=True,
                )

            # --- x prep: raw transposed operand + per-partition inv-norms ---
            xT = xp.tile([P, rows_per_core], BF16, tag="xT")
            nc.scalar.dma_start(out=xT[:], in_=x[:], transpose=True)
            x_nat = xp.tile([P, nbx, D], BF16, tag="xnat")
            # SWDGE: keeps both HWDGE rings free for transpose-loads/stores.
            nc.gpsimd.dma_start(
                out=x_nat[:], in_=x[:].rearrange("(b p) d -> p b d", p=P)
            )
            xsq = xp.tile([P, nbx, D], F32, tag="xsq")
            nc.scalar.square(xsq[:], x_nat[:])
            nx2 = xp.tile([P, nbx], F32, tag="nx2")
            nc.vector.reduce_sum(nx2[:], xsq[:], axis=mybir.AxisListType.X)
            nx = xp.tile([P, nbx], F32, tag="nx")
            nc.scalar.sqrt(nx[:], nx2[:])
            invx = xp.tile([P, nbx], F32, tag="invx")
            nc.vector.reciprocal_approx_fast(invx[:], nx[:])

            # --- y chunk prep ---
            def load_chunk(r0, cols):
                tT = ytp.tile([P, CHUNK], BF16, tag="yt")
                nc.scalar.dma_start(
                    out=tT[:, :cols], in_=y[r0 : r0 + cols, :], transpose=True
                )
                return tT

            def norm_chunk(tT, cols):
                sq = sqp.tile([P, CHUNK], BF16, tag="sq")
                nc.gpsimd.tensor_mul(sq[:, :cols], tT[:, :cols], tT[:, :cols])
                tn = ynp.tile([P, CHUNK], BF16, tag="yn")
                for h in range(0, cols, MMCOLS):
                    hc = min(MMCOLS, cols - h)
                    nps = npsum.tile([P, MMCOLS], F32)
                    for j in range(0, hc, NT):
                        nc.tensor.matmul(
                            nps[:, j : j + NT],
                            ones[:],
                            sq[:, h + j : h + j + NT],
                            start=True,
                            stop=True,
                        )
                    # 1/||y||^2 broadcast on every partition, fused PSUM drain.
                    in2 = in2p.tile([P, MMCOLS], F32, tag="in2")
                    nc.vector.reciprocal_approx_fast(in2[:, :hc], nps[:, :hc])
                    invb = invbp.tile([P, MMCOLS], BF16, tag="invb")
                    nc.scalar.sqrt(invb[:, :hc], in2[:, :hc])
                    nc.gpsimd.tensor_mul(
                        tn[:, h : h + hc], tT[:, h : h + hc], invb[:, :hc]
                    )
                return tn

            drain_rr = 0
            yTn = norm_chunk(load_chunk(0, chunk_cols[0]), chunk_cols[0])
            yT_next = None
            for c, cols in enumerate(chunk_cols):
                col0 = chunk_starts[c]
                has_next = c + 1 < len(chunk_cols)
                yTn_next = None
                for b in range(nbx):
                    if b == 0 and has_next:
                        # Next chunk's load rides ahead of the drain flood.
                        yT_next = load_chunk(chunk_starts[c + 1], chunk_cols[c + 1])
                    if b == 1 and has_next:
                        yTn_next = norm_chunk(yT_next, chunk_cols[c + 1])
                    lhs = xT[:, b * P : (b + 1) * P]
                    ob = obufp.tile([P, CHUNK], BF16, tag="ob")
                    for h in range(0, cols, MMCOLS):
                        hc = min(MMCOLS, cols - h)
                        ps = mpsum.tile([P, MMCOLS], F32)
                        for j in range(0, hc, NT):
                            nc.tensor.matmul(
                                ps[:, j : j + NT],
                                lhs,
                                yTn[:, h + j : h + j + NT],
                                start=True,
                                stop=True,
                            )
                        dst = ob[:, h : h + hc]
                        # PSUM->SBUF drain alternates DVE/ACT; the x-row
                        # inv-norm scale rides along for free on both.
                        if drain_rr % 2 == 0:
                            nc.vector.tensor_scalar_mul(
                                dst, ps[:, :hc], invx[:, b : b + 1]
                            )
                        else:
                            nc.scalar.activation(
                                dst, ps[:, :hc], ACTF.Copy, scale=invx[:, b : b + 1]
                            )
                        drain_rr += 1
                    nc.sync.dma_start(
                        out=out[b * P : (b + 1) * P, col0 : col0 + cols],
                        in_=ob[:, :cols],
                    )
                if yTn_next is not None:
                    yTn = yTn_next

    nc.finalize()
    return nc


_NC_CACHE: dict[tuple[int, int], bass.Bass] = {}


def run_spmd(input1: np.ndarray, input2: np.ndarray, **kwargs):
    """Shard, run on 8 cores, gather. Returns (output, BassKernelResults)."""
    x_bf = np.asarray(input1, dtype=np.float32).astype(ml_dtypes.bfloat16)
    y_bf = np.ascontiguousarray(
        np.asarray(input2, dtype=np.float32).astype(ml_dtypes.bfloat16)
    )
    n, d = x_bf.shape
    m, d2 = y_bf.shape
    assert d == D and d2 == D and n % N_CORES == 0
    rows = n // N_CORES

    key = (rows, m)
    if key not in _NC_CACHE:
        _NC_CACHE[key] = build_nc(rows, m)
    nc = _NC_CACHE[key]

    in_maps = [
        {"x": np.ascontiguousarray(x_bf[c * rows : (c + 1) * rows]), "y": y_bf}
        for c in range(N_CORES)
    ]
    res = run_bass_kernel_spmd(nc, in_maps, core_ids=list(range(N_CORES)), **kwargs)
    out = np.concatenate(
        [res.results[c]["out"].astype(np.float32) for c in range(N_CORES)], axis=0
    )
    return out, res


def kernel(input1: np.ndarray, input2: np.ndarray) -> np.ndarray:
    return run_spmd(input1, input2)[0]
